# revision 18
# baseline (speedup 1.0000x reference)
"""Bass/Trainium2 kernel for EnergyGNN (3-layer GIN + BN + global mean pool).

Sharding: nodes (by dst) split contiguously across 8 cores; edges assigned to
the owner of their dst node. Aggregation h[dst] += table[src] is done with
bulk dma_gather (256B rows) + one-hot matmuls on the PE (feature-major
output), MLP/BN feature-major per 128-node block, BN stats via AllReduce,
node tables all-gathered between layers.
"""
import sys
sys.path.insert(0, "/opt/trn_rl_repo")
import os
import numpy as np

import concourse.bass as bass
import concourse.bacc as bacc
import concourse.tile as tile
from concourse import mybir
from concourse.bass_utils import run_bass_kernel_spmd

# problem constants (hardcoded per harness contract)
N = 100000
E = 1600000
D = 64
NG = 128
NDEV = 8
BN_EPS = 1e-5

NPD = N // NDEV          # nodes per device = 12500
BLK = 128
NBLK = (NPD + BLK - 1) // BLK   # 98 blocks (last has 84 real nodes)
LASTBLK = NPD - (NBLK - 1) * BLK  # 84
WINROWS = 25000
NW = 4
SBLK = 4                 # blocks per superblock
CALL_CAP = 1024          # max gather indices per dma_gather call (HW ring limit)

F32 = mybir.dt.float32
I16 = mybir.dt.int16


def _host_plan(src, dst):
    """Build the shared loop structure + per-device index/onehot streams."""
    dev = dst // NPD
    blk = (dst % NPD) // BLK
    loc = (dst % NPD) % BLK
    win = np.minimum(src // WINROWS, NW - 1)

    # chunk counts per (blk, w): max over devices so program is shared
    cnt = np.zeros((NDEV, NBLK, NW), np.int64)
    np.add.at(cnt, (dev, blk, win), 1)
    chunks = (cnt + BLK - 1) // BLK          # per-device chunk need
    C = chunks.max(axis=0)                   # [NBLK, NW] shared chunk counts
    # ensure at least the segments exist when some device has edges: C already max.

    # order edges per device by (sb, w, blk, src)
    sb = blk // SBLK
    key = np.lexsort((src, blk, win, sb, dev))
    src_s, dev_s, blk_s, loc_s, win_s = (
        src[key], dev[key], blk[key], loc[key], win[key])

    # per-device segment boundaries
    seg_ptr = np.zeros((NDEV, NBLK, NW), np.int64)
    np.add.at(seg_ptr, (dev_s, blk_s, win_s), 1)  # counts again (sorted order)

    total_chunks = int(C.sum())
    total_slots = total_chunks * BLK

    idx16 = np.zeros((NDEV, total_slots), np.int16)
    dstsel = np.full((NDEV, total_chunks, BLK), -1.0, np.float32)

    # slot offsets: walk (sb, w, blk) in stream order
    chunk_meta = []  # (blk, w, is_first_chunk_of_block_group, is_last)
    # first build the global chunk layout
    sb_list = [list(range(s * SBLK, min((s + 1) * SBLK, NBLK)))
               for s in range((NBLK + SBLK - 1) // SBLK)]
    chunk_cursor = 0
    call_list = []   # (w, idx_slot_offset, n_idx, [chunk ids])
    blk_nchunks = np.zeros(NBLK, np.int64)
    for blocks in sb_list:
        for w in range(NW):
            run_chunks = []
            for b in blocks:
                for _ in range(int(C[b, w])):
                    chunk_meta.append((b, w))
                    run_chunks.append(chunk_cursor)
                    blk_nchunks[b] += 1
                    chunk_cursor += 1
            # split run into calls of <= CALL_CAP/BLK chunks
            cap = CALL_CAP // BLK
            for i in range(0, len(run_chunks), cap):
                cs = run_chunks[i:i + cap]
                call_list.append((w, cs[0] * BLK, len(cs) * BLK, cs))
    assert chunk_cursor == total_chunks

    # chunk start offset per (blk, w) per stream: compute per-device fill
    # stream slot ranges for chunk j: [j*BLK, (j+1)*BLK)
    # per (b, w) the chunks are consecutive in stream: find first chunk id
    first_chunk = {}
    cc = 0
    for blocks in sb_list:
        for w in range(NW):
            for b in blocks:
                first_chunk[(b, w)] = cc
                cc += int(C[b, w])

    # fill per-device
    edge_pos = np.zeros((NDEV,), np.int64)
    # compute per (dev, b, w) start in sorted edge array
    counts_sorted = cnt  # same counts
    # build offsets by iterating in the sorted key order groups:
    # sorted order is (dev, sb, w, blk) so do cumulative walk
    start = np.zeros((NDEV, NBLK, NW), np.int64)
    pos = 0
    for d in range(NDEV):
        for blocks in sb_list:
            for w in range(NW):
                for b in blocks:
                    start[d, b, w] = pos
                    pos += int(cnt[d, b, w])
    assert pos == E

    for d in range(NDEV):
        for b in range(NBLK):
            for w in range(NW):
                n = int(cnt[d, b, w])
                if C[b, w] == 0:
                    continue
                s0 = int(start[d, b, w])
                slot0 = first_chunk[(b, w)] * BLK
                if n > 0:
                    idx16[d, slot0:slot0 + n] = (src_s[s0:s0 + n] -
                                                 win_s[s0:s0 + n] * WINROWS)
                    fc = first_chunk[(b, w)]
                    dv = dstsel[d].reshape(-1)
                    dv[fc * BLK:fc * BLK + n] = loc_s[s0:s0 + n]
                # padding slots already idx 0 / dstsel -1

    # wrap idx16 per call into [128, n/16] layout: idx k -> [k%16, k//16],
    # replicated into all 8 groups of 16 partitions (one per Q7 cpu)
    idx_wrapped = np.zeros((NDEV, 128, total_slots // 16), np.int16)
    for (w, off, nidx, cs) in call_list:
        col0 = off // 16
        for d in range(NDEV):
            seg = idx16[d, off:off + nidx].reshape(-1, 16).T  # [16, nidx/16]
            idx_wrapped[d, :, col0:col0 + nidx // 16] = np.tile(seg, (8, 1))

    # dstsel stream as [128, total_chunks] (column per chunk)
    dstsel_cm = dstsel.transpose(0, 2, 1).copy()  # [NDEV, 128, total_chunks]

    plan = {
        "C": C, "call_list": call_list, "chunk_meta": chunk_meta,
        "sb_list": sb_list, "first_chunk": first_chunk,
        "total_chunks": total_chunks, "total_slots": total_slots,
    }
    return plan, idx_wrapped, dstsel_cm


def _build_bass(plan):
    total_chunks = plan["total_chunks"]
    total_slots = plan["total_slots"]
    call_list = plan["call_list"]
    chunk_meta = plan["chunk_meta"]
    sb_list = plan["sb_list"]
    C = plan["C"]
    NPAD = NBLK * BLK  # 12544

    nc = bacc.Bacc(None, target_bir_lowering=False, debug=False,
                   num_devices=NDEV)
    # ---- I/O ----
    t_x = nc.dram_tensor("x_table", [N, D], F32, kind="ExternalInput")
    t_xT = nc.dram_tensor("xT_own", [D, NPAD], F32, kind="ExternalInput")
    t_idx = nc.dram_tensor("idx16", [128, total_slots // 16], I16,
                           kind="ExternalInput")
    t_dst = nc.dram_tensor("dstsel", [BLK, total_chunks], F32,
                           kind="ExternalInput")
    t_pool = nc.dram_tensor("poolmat", [NBLK, BLK, BLK], F32,
                            kind="ExternalInput")
    t_iota = nc.dram_tensor("iota128", [BLK, BLK], F32, kind="ExternalInput")
    t_eye = nc.dram_tensor("eye128", [BLK, BLK], F32, kind="ExternalInput")
    wnames = []
    for l in range(1, 4):
        wnames += [f"w{l}1", f"w{l}2"]
    t_w = {k: nc.dram_tensor(k, [D, D], F32, kind="ExternalInput")
           for k in wnames}
    t_w["fw1"] = nc.dram_tensor("fw1", [D, D], F32, kind="ExternalInput")
    t_w["fw2"] = nc.dram_tensor("fw2", [D, 1], F32, kind="ExternalInput")
    vnames = []
    for l in range(1, 4):
        vnames += [f"b{l}1", f"b{l}2", f"g{l}", f"bt{l}"]
    vnames += ["fb1", "fb2"]
    t_v = {k: nc.dram_tensor(k, [D if k != "fb2" else 1], F32,
                             kind="ExternalInput")
           for k in vnames}
    t_out = nc.dram_tensor("out", [NG, 1], F32, kind="ExternalOutput")

    core_ids = list(range(NDEV))

    with tile.TileContext(nc) as tc:
        with (
            tc.tile_pool(name="big", bufs=1) as big,
            tc.tile_pool(name="const", bufs=1) as const,
            tc.tile_pool(name="gat", bufs=1) as gatp,
            tc.tile_pool(name="oh", bufs=3) as ohp,
            tc.tile_pool(name="idxp", bufs=1) as idxp,
            tc.tile_pool(name="dstp", bufs=2) as dstp,
            tc.tile_pool(name="mlp", bufs=3) as mlpp,
            tc.tile_pool(name="psum_agg", bufs=SBLK, space="PSUM") as ps_agg,
            tc.tile_pool(name="psum_mlp", bufs=2, space="PSUM") as ps_mlp,
            tc.tile_pool(name="psum_misc", bufs=1, space="PSUM") as ps_misc,
            tc.tile_pool(name="psum_pool", bufs=1, space="PSUM") as ps_pool,
            tc.tile_pool(name="dram", bufs=1, space="DRAM") as dram,
        ):
            # ---- persistent SBUF ----
            # rows 0:64 = h_ownT (feature-major current node features)
            # rows 64:128 = h2T (pre-BN activations of current layer)
            hown = big.tile([64, NPAD], F32, name="hown")
            h2t = big.tile([64, NPAD], F32, name="h2t")
            nc.sync.dma_start(out=hown[:], in_=t_xT[:])

            iota_t = const.tile([BLK, BLK], F32)
            nc.sync.dma_start(out=iota_t[:], in_=t_iota[:])
            eye_t = const.tile([BLK, BLK], F32)
            nc.sync.dma_start(out=eye_t[:], in_=t_eye[:])
            w_t = {}
            for k, tt in t_w.items():
                w_t[k] = const.tile([D, tt.shape[1]], F32, tag=f"w_{k}", name=f"w_{k}")
                nc.sync.dma_start(out=w_t[k][:], in_=tt[:])
            v_t = {}
            for k, tt in t_v.items():
                n = tt.shape[0]
                v_t[k] = const.tile([n, 1], F32, tag=f"v_{k}", name=f"v_{k}")
                nc.sync.dma_start(out=v_t[k][:], in_=tt[:, None])

            # stats / bn scratch
            stats_acc = const.tile([D, 2], F32)     # col0 = sum, col1 = sumsq
            bn_a = const.tile([D, 1], F32)
            bn_c = const.tile([D, 1], F32)
            pooled_sb = const.tile([NG, D], F32)

            # DRAM internals
            tableA = dram.tile([N, D], F32)
            tableB = dram.tile([N, D], F32)
            shard = dram.tile([NPD, D], F32)
            stats_in = dram.tile([D, 2], F32)
            stats_out = dram.tile([D, 2], F32)
            pooled_in = dram.tile([NG, D], F32)
            pooled_out = dram.tile([NG, D], F32)

            def layer(li, table_in, table_out):
                l = li + 1
                w1, w2 = w_t[f"w{l}1"], w_t[f"w{l}2"]
                b1, b2 = v_t[f"b{l}1"], v_t[f"b{l}2"]
                gg, bb = v_t[f"g{l}"], v_t[f"bt{l}"]

                nc.vector.memset(stats_acc[:], 0.0)

                # ---- pass 1: aggregate + MLP per superblock ----
                agg_ps = {}
                maxsb = int(os.environ.get("K_MAXSB", "999"))
                # iterate calls grouped per superblock
                call_i = 0
                for sbi, blocks in enumerate(sb_list):
                    if sbi >= maxsb:
                        break
                    # psum tiles for this superblock
                    for b in blocks:
                        agg_ps[b] = ps_agg.tile([D, BLK], F32, tag="agg", name=f"agg_{b}")
                    # chunk progress per block for start/stop flags
                    done = {b: 0 for b in blocks}
                    tot = {b: int(C[b].sum()) for b in blocks}
                    for w in range(NW):
                        # calls belonging to (this sb, w)
                        while call_i < len(call_list):
                            cw, off, nidx, cs = call_list[call_i]
                            if chunk_meta[cs[0]][0] not in blocks or cw != w:
                                break
                            call_i += 1
                            idx_t = idxp.tile([128, nidx // 16], I16, tag="idx")
                            nc.sync.dma_start(
                                out=idx_t[:],
                                in_=t_idx[:, off // 16:(off + nidx) // 16])
                            gat_t = gatp.tile([BLK, nidx // BLK, D], F32,
                                              tag="gat")
                            nc.gpsimd.dma_gather(
                                out_ap=gat_t[:],
                                in_ap=table_in[w * WINROWS:(w + 1) * WINROWS, :],
                                idxs_ap=idx_t[:],
                                num_idxs=nidx, num_idxs_reg=nidx,
                                elem_size=D,
                            )
                            dst_t = dstp.tile([BLK, nidx // BLK], F32,
                                              tag="dst")
                            nc.sync.dma_start(
                                out=dst_t[:],
                                in_=t_dst[:, cs[0]:cs[0] + len(cs)])
                            for j, ch in enumerate(cs):
                                b = chunk_meta[ch][0]
                                oh_t = ohp.tile([BLK, BLK], F32, tag="oh")
                                nc.vector.tensor_scalar(
                                    out=oh_t[:], in0=iota_t[:],
                                    scalar1=dst_t[:, j:j + 1], scalar2=None,
                                    op0=mybir.AluOpType.is_equal,
                                )
                                nc.tensor.matmul(
                                    out=agg_ps[b][:],
                                    lhsT=gat_t[:, j, :],
                                    rhs=oh_t[:],
                                    start=(done[b] == 0),
                                    stop=(done[b] == tot[b] - 1),
                                )
                                done[b] += 1
                    # MLP for the superblock's blocks
                    for b in blocks:
                        assert done[b] == tot[b]
                        col = b * BLK
                        zT = mlpp.tile([D, BLK], F32, tag="zT")
                        nc.vector.tensor_add(
                            out=zT[:], in0=agg_ps[b][:],
                            in1=hown[:, col:col + BLK])
                        if os.environ.get("K_AGG_ONLY"):
                            nc.vector.tensor_copy(
                                out=h2t[:, col:col + BLK], in_=zT[:])
                            continue
                        h1ps = ps_mlp.tile([D, BLK], F32, tag="mm")
                        nc.tensor.matmul(out=h1ps[:], lhsT=w1[:], rhs=zT[:],
                                         start=True, stop=True)
                        h1T = mlpp.tile([D, BLK], F32, tag="h1T")
                        nc.scalar.activation(
                            out=h1T[:], in_=h1ps[:],
                            func=mybir.ActivationFunctionType.Relu,
                            bias=b1[:], scale=1.0)
                        h2ps = ps_mlp.tile([D, BLK], F32, tag="mm")
                        nc.tensor.matmul(out=h2ps[:], lhsT=w2[:], rhs=h1T[:],
                                         start=True, stop=True)
                        nc.scalar.activation(
                            out=h2t[:, col:col + BLK], in_=h2ps[:],
                            func=mybir.ActivationFunctionType.Identity,
                            bias=b2[:], scale=1.0)
                    if os.environ.get("K_AGG_ONLY"):
                        continue
                    # stats over this superblock's h2 (mask pad nodes of
                    # the last block by restricting width)
                    c0 = blocks[0] * BLK
                    c1 = min(blocks[-1] * BLK + BLK,
                             (NBLK - 1) * BLK + LASTBLK)
                    ssum = mlpp.tile([D, 1], F32, tag="ssum")
                    nc.vector.tensor_reduce(
                        out=ssum[:], in_=h2t[:, c0:c1],
                        axis=mybir.AxisListType.X, op=mybir.AluOpType.add)
                    nc.vector.tensor_add(out=stats_acc[:, 0:1],
                                         in0=stats_acc[:, 0:1], in1=ssum[:])
                    ssq = mlpp.tile([D, 1], F32, tag="ssq")
                    sqscratch = mlpp.tile([D, c1 - c0], F32, tag="sqscratch")
                    nc.vector.tensor_mul(
                        out=sqscratch[:], in0=h2t[:, c0:c1], in1=h2t[:, c0:c1])
                    nc.vector.tensor_reduce(
                        out=ssq[:], in_=sqscratch[:],
                        axis=mybir.AxisListType.X, op=mybir.AluOpType.add)
                    nc.vector.tensor_add(out=stats_acc[:, 1:2],
                                         in0=stats_acc[:, 1:2], in1=ssq[:])

                # ---- BN stats allreduce ----
                nc.sync.dma_start(out=stats_in[:], in_=stats_acc[:])
                if os.environ.get("K_NO_COLL"):
                    bounce = mlpp.tile([D, 2], F32, tag="bounce", name="bounce")
                    nc.sync.dma_start(out=bounce[:], in_=stats_in[:])
                    nc.sync.dma_start(out=stats_out[:], in_=bounce[:])
                else:
                    nc.gpsimd.collective_compute(
                        "AllReduce", mybir.AluOpType.add,
                        replica_groups=[core_ids],
                        ins=[stats_in[:].opt()], outs=[stats_out[:].opt()])
                gstats = mlpp.tile([D, 2], F32, tag="gstats")
                nc.sync.dma_start(out=gstats[:], in_=stats_out[:])
                mean = mlpp.tile([D, 1], F32, tag="mean")
                nc.vector.tensor_scalar_mul(mean[:], gstats[:, 0:1], 1.0 / N)
                # var = ss/N - mean^2
                var = mlpp.tile([D, 1], F32, tag="var")
                nc.vector.tensor_scalar_mul(var[:], gstats[:, 1:2], 1.0 / N)
                msq = mlpp.tile([D, 1], F32, tag="msq")
                nc.vector.tensor_mul(msq[:], mean[:], mean[:])
                nc.vector.tensor_sub(var[:], var[:], msq[:])
                # a = gamma / sqrt(var + eps); c = beta - mean * a
                nc.vector.tensor_scalar_add(var[:], var[:], BN_EPS)
                nc.scalar.sqrt(var[:], var[:])
                nc.vector.reciprocal(bn_a[:], var[:])
                nc.vector.tensor_mul(bn_a[:], bn_a[:], gg[:])
                nc.vector.tensor_mul(bn_c[:], mean[:], bn_a[:])
                nc.vector.tensor_sub(bn_c[:], bb[:], bn_c[:])

                # ---- pass 2: apply BN + relu; transpose; write shard/pool ---
                if li == 2:
                    pool_ps = ps_pool.tile([NG, D], F32, tag="poolacc")
                nblk_lim = min(NBLK, maxsb * SBLK)
                if os.environ.get("K_AGG_ONLY"):
                    nblk_lim = 0
                for b in range(nblk_lim):
                    col = b * BLK
                    nrow = BLK if b < NBLK - 1 else LASTBLK
                    nc.vector.tensor_scalar(
                        out=hown[:, col:col + BLK],
                        in0=h2t[:, col:col + BLK],
                        scalar1=bn_a[:], scalar2=bn_c[:],
                        op0=mybir.AluOpType.mult, op1=mybir.AluOpType.add)
                    nc.scalar.activation(
                        out=hown[:, col:col + BLK],
                        in_=hown[:, col:col + BLK],
                        func=mybir.ActivationFunctionType.Relu)
                    hnm = mlpp.tile([BLK, D], F32, tag="hnm")
                    for bi in range(2):
                        for bj in range(4):
                            nc.vector.transpose(
                                out=hnm[bj * 32:(bj + 1) * 32,
                                        bi * 32:(bi + 1) * 32],
                                in_=hown[bi * 32:(bi + 1) * 32,
                                         col + bj * 32:col + (bj + 1) * 32])
                    if li < 2:
                        nc.sync.dma_start(
                            out=shard[col:col + nrow, :], in_=hnm[0:nrow, :])
                    else:
                        pm = dstp.tile([BLK, BLK], F32, tag="poolm")
                        nc.sync.dma_start(out=pm[:], in_=t_pool[b])
                        nc.tensor.matmul(
                            out=pool_ps[:], lhsT=pm[:], rhs=hnm[:],
                            start=(b == 0), stop=(b == nblk_lim - 1))
                if li < 2:
                    if os.environ.get("K_NO_COLL"):
                        pass
                    else:
                        nc.gpsimd.collective_compute(
                            "AllGather", mybir.AluOpType.bypass,
                            replica_groups=[core_ids],
                            ins=[shard[:].opt()], outs=[table_out[:].opt()])
                else:
                    nc.scalar.copy(out=pooled_sb[:], in_=pool_ps[:])

            nlayers = int(os.environ.get("K_LAYERS", "3"))
            layer(0, t_x, tableA)
            if nlayers > 1:
                layer(1, tableA, tableB)
            if nlayers > 2:
                layer(2, tableB, None)
            if nlayers <= 2:
                nc.scalar.copy(out=pooled_sb[:], in_=eye_t[:, 0:64])

            # ---- pooled allreduce + final MLP ----
            nc.sync.dma_start(out=pooled_in[:], in_=pooled_sb[:])
            if os.environ.get("K_NO_COLL"):
                bounce2 = mlpp.tile([NG, D], F32, tag="bounce2", name="bounce2")
                nc.sync.dma_start(out=bounce2[:], in_=pooled_in[:])
                nc.sync.dma_start(out=pooled_out[:], in_=bounce2[:])
            else:
                nc.gpsimd.collective_compute(
                    "AllReduce", mybir.AluOpType.add,
                    replica_groups=[core_ids],
                    ins=[pooled_in[:].opt()], outs=[pooled_out[:].opt()])
            pooled_t = mlpp.tile([NG, D], F32, tag="pooledf")
            nc.sync.dma_start(out=pooled_t[:], in_=pooled_out[:])
            pT = mlpp.tile([D, NG], F32, tag="pT")
            for bi in range(4):
                for bj in range(2):
                    nc.vector.transpose(
                        out=pT[bj * 32:(bj + 1) * 32, bi * 32:(bi + 1) * 32],
                        in_=pooled_t[bi * 32:(bi + 1) * 32,
                                     bj * 32:(bj + 1) * 32])
            f1ps = ps_mlp.tile([D, NG], F32, tag="mm")
            nc.tensor.matmul(out=f1ps[:], lhsT=w_t["fw1"][:], rhs=pT[:],
                             start=True, stop=True)
            f1 = mlpp.tile([D, NG], F32, tag="f1")
            nc.scalar.activation(out=f1[:], in_=f1ps[:],
                                 func=mybir.ActivationFunctionType.Relu,
                                 bias=v_t["fb1"][:], scale=1.0)
            f2ps = ps_mlp.tile([1, NG], F32, tag="mm")
            nc.tensor.matmul(out=f2ps[:], lhsT=w_t["fw2"][:], rhs=f1[:],
                             start=True, stop=True)
            f2 = mlpp.tile([1, NG], F32, tag="f2")
            nc.scalar.activation(out=f2[:], in_=f2ps[:],
                                 func=mybir.ActivationFunctionType.Relu,
                                 bias=v_t["fb2"][:], scale=1.0)
            nc.sync.dma_start(out=t_out[:, 0:1], in_=f2[0:1, :])

    nc.compile()
    return nc


def _host_inputs(inputs, plan, idx_wrapped, dstsel_cm):
    x = np.asarray(inputs["x"], np.float32)
    edge_index = np.asarray(inputs["edge_index"])
    batch = np.asarray(inputs["batch"]).astype(np.int64)

    cnts = np.bincount(batch, minlength=NG).astype(np.float32)
    inv = 1.0 / np.maximum(cnts, 1.0)

    iota = np.tile(np.arange(BLK, dtype=np.float32)[None, :], (BLK, 1))
    eye = np.eye(BLK, dtype=np.float32)

    in_maps = []
    for d in range(NDEV):
        nodes = np.arange(d * NPD, (d + 1) * NPD)
        xT = np.zeros((D, NBLK * BLK), np.float32)
        xT[:, :NPD] = x[nodes].T
        poolmat = np.zeros((NBLK, BLK, BLK), np.float32)
        g = batch[nodes]
        for i in range(NPD):
            poolmat[i // BLK, i % BLK, g[i]] = inv[g[i]]
        m = {
            "x_table": x,
            "xT_own": xT,
            "idx16": idx_wrapped[d],
            "dstsel": dstsel_cm[d],
            "poolmat": poolmat,
            "iota128": iota,
            "eye128": eye,
        }
        for k in ["w11", "w12", "w21", "w22", "w31", "w32", "fw1", "fw2",
                  "b11", "b12", "g1", "bt1", "b21", "b22", "g2", "bt2",
                  "b31", "b32", "g3", "bt3", "fb1", "fb2"]:
            m[k] = np.asarray(inputs[k], np.float32)
        in_maps.append(m)
    return in_maps


_CACHE = {}


def kernel(**inputs):
    edge_index = np.asarray(inputs["edge_index"])
    src = edge_index[0].astype(np.int64)
    dst = edge_index[1].astype(np.int64)

    plan, idx_wrapped, dstsel_cm = _host_plan(src, dst)
    nc = _build_bass(plan)
    in_maps = _host_inputs(inputs, plan, idx_wrapped, dstsel_cm)
    res = run_bass_kernel_spmd(nc, in_maps, core_ids=list(range(NDEV)))
    return res.results[0]["out"].astype(np.float32)
